# revision 55
# baseline (speedup 1.0000x reference)
"""Trainium2 Bass kernel for single-head attention.

Problem: x[8, 2048, 512], Wq/Wk/Wv[512, 512], bq/bk/bv[512] ->
out[8, 2048, 512] where out = softmax((xWq+bq)(xWk+bk)^T / sqrt(512)) (xWv+bv).

Sharding: data-parallel over batch; each NeuronCore does one batch element.

Key ideas (vs the straightforward fp32r kernel):

1. Algebraic folding. q.k^T = x (Wq Wk^T) x^T + (x Wq).bk + bq.(x Wk) + bq.bk.
   Terms constant along the softmax (key) axis cancel in softmax, so only
   M' = Wq Wk^T * scale (host-precomputed) and the per-key bias
   b_j = x_j . (Wk bq * scale) survive. The Q and K projections collapse into
   ONE projection kT = M' x^T, and eT[j, i] = exp(x_i . kT_j + b_j - SHIFT).
   b_j rides in the ACT per-partition bias of the exp; the e^{-SHIFT} and
   per-query factors cancel in num/den.

2. fp8 DoubleRow matmuls. TRN2's fp8e4 DoubleRow mode runs 2 stacked
   128-contraction slabs per instruction at 0.5 cycles/row -> 4x fp32r
   MAC throughput. e4m3 alone is far too coarse (~2.5% per element), so every
   operand is split hi/lo: a = fp8(a) + fp8(a - fp8(a)), exact to ~7e-4, and
   each product uses 3 of the 4 cross terms (ah.bh + al.bh + ah.bl; the
   dropped al.bl is ~7e-4 relative). Host splits x^T, M', Wv exactly; the
   device splits its own intermediates (kT, v, e) with one fp8 copy + one
   subtract per tile, spread across ACT/DVE/Pool so no engine exceeds ~70us
   (Pool runs at 0.42-0.6 efficiency and cannot touch PSUM). e4m3's narrow
   range ([2^-9, 240]) forces power-of-2 pre-scales A_M/A_3/A_V on M'/m3/Wv,
   compensated for free in the exp scale, the bias-prep scale, and the
   fp8 value of the denominator 'ones' vector.

3. Denominators and the bias row are computed TRANSPOSED as 1-column
   DoubleRow matmuls whose stationary operands are shared with the adjacent
   attn*V / v-projection instructions, so they cost ~0 PE cycles and need no
   [1,512]->[128,..] transposes. bv is applied at output evacuation with a
   fused (psum * 1/den) + bv scalar_tensor_tensor. No PE transposes of x
   (host ships x^T pre-split).

Per-core matmul cycle budget (PE at 2.4GHz): kT-proj 24.6K, v-proj 24.6K,
scores 98.3K, attn*V 98.3K, den/bias ~0.2K => ~246K cycles ~= 103us, vs
379K (~158us) for the fp32r baseline. Measured rel err ~2.4e-3 (budget 2e-2).
"""

import os
import sys

for _p in ("/opt/trn_rl_repo", "/root/.axon_site/_ro/trn_rl_repo"):
    if os.path.isdir(_p) and _p not in sys.path:
        sys.path.append(_p)

import numpy as np
import ml_dtypes

import concourse.bacc as bacc
import concourse.mybir as mybir
import concourse.tile as tile
from concourse.bass_utils import run_bass_kernel_spmd

B = 8
S = 2048
D = 512
P = 128
NT = S // P  # 16 key tiles
MC = 4  # query chunks of 512
SCALE = 1.0 / float(np.sqrt(D))
SHIFT = 3.0  # keeps exp(score - SHIFT) well inside fp8e4 range (max ~240)
# Power-of-2 pre-scales keep each fp8 operand mid-range (e4m3 spans only
# [2^-9, 240]); they are exact in fp8 and compensated downstream: A_M in the
# exp's scale, A_3 in the bias prep, A_V in the den 'ones' fp8 value.
A_M = 512.0
A_3 = 8192.0
A_V = 32.0

F32 = mybir.dt.float32
F8 = mybir.dt.float8e4
E4M3 = ml_dtypes.float8_e4m3
ACT_EXP = mybir.ActivationFunctionType.Exp
ACT_ID = mybir.ActivationFunctionType.Identity
DR = mybir.MatmulPerfMode.DoubleRow
MULT = mybir.AluOpType.mult
ADD = mybir.AluOpType.add

_CACHE = {}


def _split8(a):
    """Split fp32 array into (hi, lo) e4m3 parts with hi + lo ~= a."""
    hi = a.astype(E4M3)
    lo = (a - hi.astype(np.float32)).astype(E4M3)
    return hi, lo


def _build_nc():
    nc = bacc.Bacc(None)

    xh = nc.dram_tensor("xh", [D, S], F8, kind="ExternalInput")
    xl = nc.dram_tensor("xl", [D, S], F8, kind="ExternalInput")
    mht = nc.dram_tensor("mht", [D, D], F8, kind="ExternalInput")
    mlt = nc.dram_tensor("mlt", [D, D], F8, kind="ExternalInput")
    wvh = nc.dram_tensor("wvh", [D, D], F8, kind="ExternalInput")
    wvl = nc.dram_tensor("wvl", [D, D], F8, kind="ExternalInput")
    m3 = nc.dram_tensor("m3", [D, 1], F8, kind="ExternalInput")
    bv = nc.dram_tensor("bv", [D], F32, kind="ExternalInput")
    out = nc.dram_tensor("out", [S, D], F32, kind="ExternalOutput")

    with tile.TileContext(nc) as tc:
        with (
            tc.tile_pool(name="consts", bufs=1) as consts,
            tc.tile_pool(name="xT", bufs=1) as xT_pool,
            tc.tile_pool(name="wsb", bufs=1) as w_pool,
            tc.tile_pool(name="ksb", bufs=1) as k_pool,
            tc.tile_pool(name="vsb", bufs=1) as v_pool,
            tc.tile_pool(name="e32", bufs=7) as e32_pool,
            tc.tile_pool(name="e8", bufs=7) as e8_pool,
            tc.tile_pool(name="den", bufs=2) as den_pool,
            tc.tile_pool(name="outsb", bufs=4) as out_pool,
        ):
            # ---- constants ----
            alpha32 = consts.tile([P, 2, 1], F32, tag="alpha32", name="alpha32")
            nc.vector.memset(alpha32[:], A_V)
            alpha8 = consts.tile([P, 2, 1], F8, tag="alpha8", name="alpha8")
            nc.gpsimd.tensor_copy(alpha8[:], alpha32[:])
            neg_shift = consts.tile([P, 1], F32, tag="nshift", name="neg_shift")
            nc.vector.memset(neg_shift[:], -SHIFT)

            warm8 = consts.tile([P, 2, 512], F8, tag="warm8", name="warm8")
            nc.vector.memset(warm8[:], 1.0)

            bv_sb = consts.tile([P, D], F32, tag="bv", name="bv_sb")
            nc.scalar.dma_start(out=bv_sb[:], in_=bv[:].partition_broadcast(P))

            # m3 moving tiles [f' 128, 2, 1] per f'-pair
            m3t = []
            for c2 in range(2):
                t = consts.tile([P, 2, 1], F8, tag=f"m3_{c2}", name=f"m3_{c2}")
                nc.scalar.dma_start(
                    out=t[:],
                    in_=m3[c2 * 256 : (c2 + 1) * 256, :].rearrange(
                        "(two p) c -> p two c", p=P
                    ),
                )
                m3t.append(t)

            # weights: stationary/moving tiles [128, 2, 512] per f'-pair.
            # mh goes on the sync queue (needed first); the rest via gpsimd.
            def _wtiles(name, dram, eng):
                ts = []
                for c2 in range(2):
                    t = w_pool.tile(
                        [P, 2, D], F8, tag=f"{name}{c2}", name=f"{name}{c2}"
                    )
                    eng.dma_start(
                        out=t[:],
                        in_=dram[c2 * 256 : (c2 + 1) * 256, :].rearrange(
                            "(two p) h -> p two h", p=P
                        ),
                    )
                    ts.append(t)
                return ts

            mh_p = _wtiles("mh", mht, nc.scalar)
            ml_p = _wtiles("ml", mlt, nc.scalar)

            # x^T hi/lo, [128, 2, 2048] per f-pair. DMA'd in key-chunk
            # slices, first chunk of every tile first, so the kT projection
            # (which walks key-chunks) unblocks after ~1/4 of the bytes.
            def _xalloc(name):
                return [
                    xT_pool.tile([P, 2, S], F8, tag=f"{name}{c2}", name=f"{name}{c2}")
                    for c2 in range(2)
                ]

            xh_p = _xalloc("xh")
            xl_p = _xalloc("xl")
            for kc in range(4):
                ksl = slice(kc * 512, (kc + 1) * 512)
                for tiles, dram, q in ((xl_p, xl, nc.gpsimd), (xh_p, xh, nc.sync)):
                    for c2 in range(2):
                        q.dma_start(
                            out=tiles[c2][:, :, ksl],
                            in_=dram[c2 * 256 : (c2 + 1) * 256, ksl].rearrange(
                                "(two p) s -> p two s", p=P
                            ),
                        )

            # remaining weights follow the xl slices on the gpsimd queue
            wvh_p = _wtiles("wvh", wvh, nc.gpsimd)
            wvl_p = _wtiles("wvl", wvl, nc.gpsimd)

            # kT hi/lo [128, 2, 2048] per fhat-pair; v hi/lo [128, 16, 512]
            kh_p = [
                k_pool.tile([P, 2, S], F8, tag=f"kh{i}", name=f"kh{i}")
                for i in range(2)
            ]
            kl_p = [
                k_pool.tile([P, 2, S], F8, tag=f"kl{i}", name=f"kl{i}")
                for i in range(2)
            ]
            vh = v_pool.tile([P, NT, D], F8, tag="vh", name="vh")
            vl = v_pool.tile([P, NT, D], F8, tag="vl", name="vl")

            bias_sb = consts.tile([P, NT], F32, tag="bias", name="bias_sb")

            with (
                tc.tile_pool(name="psB", bufs=1, space="PSUM") as psB,
                tc.tile_pool(name="psProj", bufs=5, space="PSUM") as psProj,
            ):
                # ---- PE warmup: ~3.5us of junk matmuls while the x/M DMAs
                # land, so the PE p-state is at full clock for real work ----
                ps_w = psB.tile([16, 512], F32, tag="warm", name="ps_warm")
                for w in range(10):
                    nc.tensor.matmul(
                        ps_w[:],
                        warm8[:, :, 0:16],
                        warm8[:],
                        start=(w == 0),
                        stop=(w == 13),
                        perf_mode=DR,
                    )

                # ---- kT projection: kT = M'-slabs . x^T (3-combo fp8) ----
                for kc in range(4):
                    for ft in range(4):
                        ps = psProj.tile(
                            [P, 512], F32, tag="ps_p", name=f"ps_k{kc}_{ft}"
                        )
                        combos = (
                            (mh_p, xh_p),
                            (ml_p, xh_p),
                            (mh_p, xl_p),
                        )
                        n = 0
                        for wm, xm in combos:
                            for c2 in range(2):
                                nc.tensor.matmul(
                                    ps[:],
                                    wm[c2][:, :, ft * P : (ft + 1) * P],
                                    xm[c2][:, :, kc * 512 : (kc + 1) * 512],
                                    start=(n == 0),
                                    stop=(n == 5),
                                    perf_mode=DR,
                                )
                                n += 1
                        dst = (slice(None), ft % 2, slice(kc * 512, (kc + 1) * 512))
                        nc.scalar.copy(kh_p[ft // 2][dst], ps[:])
                        nc.vector.tensor_sub(
                            kl_p[ft // 2][dst], ps[:], kh_p[ft // 2][dst]
                        )

                # ---- v projection (3-combo fp8), bias bv at output ----
                # The per-key bias row b^T rides along: 1-column DoubleRow
                # matmuls reusing xh as stationary, m3 as moving.
                ps_b2 = psB.tile([P, NT], F32, tag="ps_b2", name="ps_b2")
                for st in range(NT):
                    ps = psProj.tile([P, 512], F32, tag="ps_p", name=f"ps_v{st}")
                    combos = (
                        (xh_p, wvh_p),
                        (xl_p, wvh_p),
                        (xh_p, wvl_p),
                    )
                    n = 0
                    for xm, wm in combos:
                        for c2 in range(2):
                            nc.tensor.matmul(
                                ps[:],
                                xm[c2][:, :, st * P : (st + 1) * P],
                                wm[c2][:],
                                start=(n == 0),
                                stop=(n == 5),
                                perf_mode=DR,
                            )
                            n += 1
                    # one accumulation group for the whole ps_b2 bank: start
                    # marks the full 2KB zero region, later columns overwrite
                    # their pending-zero bytes on first touch
                    for c2 in range(2):
                        nc.tensor.matmul(
                            ps_b2[:, st : st + 1],
                            xh_p[c2][:, :, st * P : (st + 1) * P],
                            m3t[c2][:],
                            start=(st == 0 and c2 == 0),
                            stop=(st == NT - 1 and c2 == 1),
                            perf_mode=DR,
                        )
                    nc.scalar.copy(vh[:, st, :], ps[:])
                    nc.vector.tensor_sub(vl[:, st, :], ps[:], vh[:, st, :])

                nc.scalar.activation(
                    bias_sb[:],
                    ps_b2[:],
                    ACT_ID,
                    bias=neg_shift[:],
                    scale=1.0 / A_3,
                )

            # ---- attention: scores^T -> exp -> e hi/lo -> den + attn*V ----
            # Software-pipelined: the attn*V + denominator matmuls for key
            # pair p are emitted AFTER the scores of pair p+1, so the PE
            # (in-order) never waits on the exp -> eh -> el chain (~2.4us).
            with (
                tc.tile_pool(name="psS", bufs=2, space="PSUM") as psS,
                tc.tile_pool(name="psO", bufs=1, space="PSUM") as psO,
                tc.tile_pool(name="psDen", bufs=1, space="PSUM") as psDen,
            ):
                ps_o_m = {}
                ps_den_m = {}
                pend = []

                def emit_av(m, p, eh_t, el_t):
                    ps_o = ps_o_m[m]
                    ps_den = ps_den_m[m]
                    vsl = slice(2 * p, 2 * p + 2)
                    last = p == NT // 2 - 1
                    den_qts = range(4) if last else ()
                    if last:
                        # final pair: denominators first so the reciprocal
                        # starts while the PE still runs the last AV matmuls
                        for qt in range(4):
                            qsl = slice(qt * P, (qt + 1) * P)
                            nc.tensor.matmul(
                                ps_den[:, qt : qt + 1], eh_t[:, :, qsl],
                                alpha8[:], start=False, stop=False,
                                perf_mode=DR,
                            )
                            nc.tensor.matmul(
                                ps_den[:, qt : qt + 1], el_t[:, :, qsl],
                                alpha8[:], start=False, stop=(qt == 3),
                                perf_mode=DR,
                            )
                        rec = den_pool.tile([P, 4], F32, tag="rec", name=f"rec{m}")
                        nc.vector.reciprocal(rec[:], ps_den[:])
                    for qt in range(4):
                        qsl = slice(qt * P, (qt + 1) * P)
                        for ci, (em, vm) in enumerate(
                            ((eh_t, vh), (el_t, vh), (eh_t, vl))
                        ):
                            nc.tensor.matmul(
                                ps_o[qt][:],
                                em[:, :, qsl],
                                vm[:, vsl, :],
                                start=(p == 0 and ci == 0),
                                stop=(last and ci == 2),
                                perf_mode=DR,
                            )
                        if not last:
                            # denominators ride the same stationaries, one
                            # psum group for the whole ps_den bank (start
                            # marks the full zero region; columns fill on
                            # first touch)
                            nc.tensor.matmul(
                                ps_den[:, qt : qt + 1],
                                eh_t[:, :, qsl],
                                alpha8[:],
                                start=(p == 0 and qt == 0),
                                stop=False,
                                perf_mode=DR,
                            )
                            nc.tensor.matmul(
                                ps_den[:, qt : qt + 1],
                                el_t[:, :, qsl],
                                alpha8[:],
                                start=False,
                                stop=False,
                                perf_mode=DR,
                            )
                        else:
                            # out = ps_o * (1/den) + bv, fused; DMA out
                            o_sb = out_pool.tile(
                                [P, D], F32, tag="osb", name=f"o{m}_{qt}"
                            )
                            nc.vector.scalar_tensor_tensor(
                                o_sb[:], ps_o[qt][:], rec[:, qt : qt + 1],
                                bv_sb[:], MULT, ADD,
                            )
                            it = m * 4 + qt
                            q = nc.sync if qt % 2 == 0 else nc.gpsimd
                            q.dma_start(
                                out=out[it * P : (it + 1) * P, :], in_=o_sb[:]
                            )

                for m in range(MC):
                    msl = slice(m * 512, (m + 1) * 512)
                    ps_o_m[m] = [
                        psO.tile([P, D], F32, tag=f"o{t}", name=f"ps_o{t}_{m}")
                        for t in range(4)
                    ]
                    # transposed denominators: [query 128, qt 4], scaled by A_V
                    ps_den_m[m] = psDen.tile(
                        [P, 4], F32, tag="ps_den", name=f"ps_den{m}"
                    )
                    eh_t = el_t = None
                    for kt in range(NT):
                        p = kt // 2
                        ps_s = psS.tile(
                            [P, 512], F32, tag="ps_s", name=f"ps_s{m}_{kt}"
                        )
                        ktsl = slice(kt * P, (kt + 1) * P)
                        combos = (
                            (kh_p, xh_p),
                            (kh_p, xl_p),
                            (kl_p, xh_p),
                        )
                        n = 0
                        for km, xm in combos:
                            for c2 in range(2):
                                nc.tensor.matmul(
                                    ps_s[:],
                                    km[c2][:, :, ktsl],
                                    xm[c2][:, :, msl],
                                    start=(n == 0),
                                    stop=(n == 5),
                                    perf_mode=DR,
                                )
                                n += 1
                        if kt % 2 == 0:
                            eh_t = e8_pool.tile(
                                [P, 2, 512], F8, tag="eh", name=f"eh{m}_{p}"
                            )
                            el_t = e8_pool.tile(
                                [P, 2, 512], F8, tag="el", name=f"el{m}_{p}"
                            )
                        e32 = e32_pool.tile(
                            [P, 512], F32, tag="e32", name=f"e32_{m}_{kt}"
                        )
                        nc.scalar.activation(
                            e32[:],
                            ps_s[:],
                            ACT_EXP,
                            bias=bias_sb[:, kt : kt + 1],
                            scale=1.0 / A_M,
                        )
                        # e hi/lo split: eh on Pool; el mostly on DVE with a
                        # quarter on Pool (Pool can't reach PSUM, so it only
                        # ever sees these SBUF-only ops)
                        nc.gpsimd.tensor_copy(eh_t[:, kt % 2, :], e32[:])
                        nc.vector.tensor_sub(
                            el_t[:, kt % 2, :], e32[:], eh_t[:, kt % 2, :]
                        )

                        if kt % 2 == 1:
                            pend.append((m, p, eh_t, el_t))
                            if len(pend) > 3:
                                emit_av(*pend.pop(0))
                for args in pend:
                    emit_av(*args)

    nc.finalize()
    return nc


def kernel(x, Wq, bq, Wk, bk, Wv, bv):
    x = np.asarray(x, dtype=np.float32)
    Wq = np.asarray(Wq, dtype=np.float32)
    bq = np.asarray(bq, dtype=np.float32)
    Wk = np.asarray(Wk, dtype=np.float32)
    bk = np.asarray(bk, dtype=np.float32)
    Wv = np.asarray(Wv, dtype=np.float32)
    bv = np.asarray(bv, dtype=np.float32)

    # host-side folding: stationary M' = Wk Wq^T * scale, m3 = Wk bq * scale
    mt = (
        (Wk.astype(np.float64) @ Wq.astype(np.float64).T) * SCALE * A_M
    ).astype(np.float32)
    mht, mlt = _split8(mt)
    wvh, wvl = _split8(Wv * np.float32(A_V))
    m3v = (
        (Wk.astype(np.float64) @ bq.astype(np.float64)) * SCALE * A_3
    ).astype(np.float32).astype(E4M3)
    m3 = m3v[:, None]

    shared = {
        "mht": np.ascontiguousarray(mht),
        "mlt": np.ascontiguousarray(mlt),
        "wvh": np.ascontiguousarray(wvh),
        "wvl": np.ascontiguousarray(wvl),
        "m3": np.ascontiguousarray(m3),
        "bv": np.ascontiguousarray(bv),
    }

    if "nc" not in _CACHE:
        _CACHE["nc"] = _build_nc()
    nc = _CACHE["nc"]

    in_maps = []
    for b in range(B):
        xT = np.ascontiguousarray(x[b].T)
        xhb, xlb = _split8(xT)
        in_maps.append(
            {"xh": np.ascontiguousarray(xhb), "xl": np.ascontiguousarray(xlb), **shared}
        )
    try:
        res = run_bass_kernel_spmd(nc, in_maps, list(range(B)))
    except Exception:
        # transient device wedge (e.g. NRT_EXEC_UNIT_UNRECOVERABLE) - retry
        import time as _time

        _time.sleep(5)
        res = run_bass_kernel_spmd(nc, in_maps, list(range(B)))
    return np.stack([res.results[b]["out"] for b in range(B)]).astype(np.float32)


if __name__ == "__main__":
    rng = np.random.default_rng(0)
    inputs = {
        "x": rng.standard_normal((B, S, D), dtype=np.float32),
        "Wq": rng.standard_normal((D, D), dtype=np.float32) / np.sqrt(D),
        "bq": rng.standard_normal(D).astype(np.float32) * 0.01,
        "Wk": rng.standard_normal((D, D), dtype=np.float32) / np.sqrt(D),
        "bk": rng.standard_normal(D).astype(np.float32) * 0.01,
        "Wv": rng.standard_normal((D, D), dtype=np.float32) / np.sqrt(D),
        "bv": rng.standard_normal(D).astype(np.float32) * 0.01,
    }
    got = kernel(**inputs)
    print("kernel output", got.shape, got.dtype)


# revision 56
# speedup vs baseline: 1.0011x; 1.0011x over previous
"""Trainium2 Bass kernel for single-head attention.

Problem: x[8, 2048, 512], Wq/Wk/Wv[512, 512], bq/bk/bv[512] ->
out[8, 2048, 512] where out = softmax((xWq+bq)(xWk+bk)^T / sqrt(512)) (xWv+bv).

Sharding: data-parallel over batch; each NeuronCore does one batch element.

Key ideas (vs the straightforward fp32r kernel):

1. Algebraic folding. q.k^T = x (Wq Wk^T) x^T + (x Wq).bk + bq.(x Wk) + bq.bk.
   Terms constant along the softmax (key) axis cancel in softmax, so only
   M' = Wq Wk^T * scale (host-precomputed) and the per-key bias
   b_j = x_j . (Wk bq * scale) survive. The Q and K projections collapse into
   ONE projection kT = M' x^T, and eT[j, i] = exp(x_i . kT_j + b_j - SHIFT).
   b_j rides in the ACT per-partition bias of the exp; the e^{-SHIFT} and
   per-query factors cancel in num/den.

2. fp8 DoubleRow matmuls. TRN2's fp8e4 DoubleRow mode runs 2 stacked
   128-contraction slabs per instruction at 0.5 cycles/row -> 4x fp32r
   MAC throughput. e4m3 alone is far too coarse (~2.5% per element), so every
   operand is split hi/lo: a = fp8(a) + fp8(a - fp8(a)), exact to ~7e-4, and
   each product uses 3 of the 4 cross terms (ah.bh + al.bh + ah.bl; the
   dropped al.bl is ~7e-4 relative). Host splits x^T, M', Wv exactly; the
   device splits its own intermediates (kT, v, e) with one fp8 copy + one
   subtract per tile, spread across ACT/DVE/Pool so no engine exceeds ~70us
   (Pool runs at 0.42-0.6 efficiency and cannot touch PSUM). e4m3's narrow
   range ([2^-9, 240]) forces power-of-2 pre-scales A_M/A_3/A_V on M'/m3/Wv,
   compensated for free in the exp scale, the bias-prep scale, and the
   fp8 value of the denominator 'ones' vector.

3. Denominators and the bias row are computed TRANSPOSED as 1-column
   DoubleRow matmuls whose stationary operands are shared with the adjacent
   attn*V / v-projection instructions, so they cost ~0 PE cycles and need no
   [1,512]->[128,..] transposes. bv is applied at output evacuation with a
   fused (psum * 1/den) + bv scalar_tensor_tensor. No PE transposes of x
   (host ships x^T pre-split).

Per-core matmul cycle budget (PE at 2.4GHz): kT-proj 24.6K, v-proj 24.6K,
scores 98.3K, attn*V 98.3K, den/bias ~0.2K => ~246K cycles ~= 103us, vs
379K (~158us) for the fp32r baseline. Measured rel err ~2.4e-3 (budget 2e-2).
"""

import os
import sys

for _p in ("/opt/trn_rl_repo", "/root/.axon_site/_ro/trn_rl_repo"):
    if os.path.isdir(_p) and _p not in sys.path:
        sys.path.append(_p)

import numpy as np
import ml_dtypes

import concourse.bacc as bacc
import concourse.mybir as mybir
import concourse.tile as tile
from concourse.bass_utils import run_bass_kernel_spmd

B = 8
S = 2048
D = 512
P = 128
NT = S // P  # 16 key tiles
MC = 4  # query chunks of 512
SCALE = 1.0 / float(np.sqrt(D))
SHIFT = 3.0  # keeps exp(score - SHIFT) well inside fp8e4 range (max ~240)
# Power-of-2 pre-scales keep each fp8 operand mid-range (e4m3 spans only
# [2^-9, 240]); they are exact in fp8 and compensated downstream: A_M in the
# exp's scale, A_3 in the bias prep, A_V in the den 'ones' fp8 value.
A_M = 512.0
A_3 = 8192.0
A_V = 32.0

F32 = mybir.dt.float32
F8 = mybir.dt.float8e4
E4M3 = ml_dtypes.float8_e4m3
ACT_EXP = mybir.ActivationFunctionType.Exp
ACT_ID = mybir.ActivationFunctionType.Identity
DR = mybir.MatmulPerfMode.DoubleRow
MULT = mybir.AluOpType.mult
ADD = mybir.AluOpType.add

_CACHE = {}


def _split8(a):
    """Split fp32 array into (hi, lo) e4m3 parts with hi + lo ~= a."""
    hi = a.astype(E4M3)
    lo = (a - hi.astype(np.float32)).astype(E4M3)
    return hi, lo


def _build_nc():
    nc = bacc.Bacc(None)

    xh = nc.dram_tensor("xh", [D, S], F8, kind="ExternalInput")
    xl = nc.dram_tensor("xl", [D, S], F8, kind="ExternalInput")
    mht = nc.dram_tensor("mht", [D, D], F8, kind="ExternalInput")
    mlt = nc.dram_tensor("mlt", [D, D], F8, kind="ExternalInput")
    wvh = nc.dram_tensor("wvh", [D, D], F8, kind="ExternalInput")
    wvl = nc.dram_tensor("wvl", [D, D], F8, kind="ExternalInput")
    m3 = nc.dram_tensor("m3", [D, 1], F8, kind="ExternalInput")
    bv = nc.dram_tensor("bv", [D], F32, kind="ExternalInput")
    out = nc.dram_tensor("out", [S, D], F32, kind="ExternalOutput")

    with tile.TileContext(nc) as tc:
        with (
            tc.tile_pool(name="consts", bufs=1) as consts,
            tc.tile_pool(name="xT", bufs=1) as xT_pool,
            tc.tile_pool(name="wsb", bufs=1) as w_pool,
            tc.tile_pool(name="ksb", bufs=1) as k_pool,
            tc.tile_pool(name="vsb", bufs=1) as v_pool,
            tc.tile_pool(name="e32", bufs=7) as e32_pool,
            tc.tile_pool(name="e8", bufs=7) as e8_pool,
            tc.tile_pool(name="den", bufs=2) as den_pool,
            tc.tile_pool(name="outsb", bufs=4) as out_pool,
        ):
            # ---- constants ----
            alpha32 = consts.tile([P, 2, 1], F32, tag="alpha32", name="alpha32")
            nc.vector.memset(alpha32[:], A_V)
            alpha8 = consts.tile([P, 2, 1], F8, tag="alpha8", name="alpha8")
            nc.gpsimd.tensor_copy(alpha8[:], alpha32[:])
            neg_shift = consts.tile([P, 1], F32, tag="nshift", name="neg_shift")
            nc.vector.memset(neg_shift[:], -SHIFT)

            warm8 = consts.tile([P, 2, 512], F8, tag="warm8", name="warm8")
            nc.gpsimd.memset(warm8[:], 1.0)

            bv_sb = consts.tile([P, D], F32, tag="bv", name="bv_sb")
            nc.scalar.dma_start(out=bv_sb[:], in_=bv[:].partition_broadcast(P))

            # m3 moving tiles [f' 128, 2, 1] per f'-pair
            m3t = []
            for c2 in range(2):
                t = consts.tile([P, 2, 1], F8, tag=f"m3_{c2}", name=f"m3_{c2}")
                nc.scalar.dma_start(
                    out=t[:],
                    in_=m3[c2 * 256 : (c2 + 1) * 256, :].rearrange(
                        "(two p) c -> p two c", p=P
                    ),
                )
                m3t.append(t)

            # weights: stationary/moving tiles [128, 2, 512] per f'-pair.
            # mh goes on the sync queue (needed first); the rest via gpsimd.
            def _wtiles(name, dram, eng):
                ts = []
                for c2 in range(2):
                    t = w_pool.tile(
                        [P, 2, D], F8, tag=f"{name}{c2}", name=f"{name}{c2}"
                    )
                    eng.dma_start(
                        out=t[:],
                        in_=dram[c2 * 256 : (c2 + 1) * 256, :].rearrange(
                            "(two p) h -> p two h", p=P
                        ),
                    )
                    ts.append(t)
                return ts

            mh_p = _wtiles("mh", mht, nc.scalar)
            ml_p = _wtiles("ml", mlt, nc.scalar)

            # x^T hi/lo, [128, 2, 2048] per f-pair. DMA'd in key-chunk
            # slices, first chunk of every tile first, so the kT projection
            # (which walks key-chunks) unblocks after ~1/4 of the bytes.
            def _xalloc(name):
                return [
                    xT_pool.tile([P, 2, S], F8, tag=f"{name}{c2}", name=f"{name}{c2}")
                    for c2 in range(2)
                ]

            xh_p = _xalloc("xh")
            xl_p = _xalloc("xl")
            for kc in range(4):
                ksl = slice(kc * 512, (kc + 1) * 512)
                for tiles, dram, q in ((xl_p, xl, nc.gpsimd), (xh_p, xh, nc.sync)):
                    for c2 in range(2):
                        q.dma_start(
                            out=tiles[c2][:, :, ksl],
                            in_=dram[c2 * 256 : (c2 + 1) * 256, ksl].rearrange(
                                "(two p) s -> p two s", p=P
                            ),
                        )

            # remaining weights follow the xl slices on the gpsimd queue
            wvh_p = _wtiles("wvh", wvh, nc.gpsimd)
            wvl_p = _wtiles("wvl", wvl, nc.gpsimd)

            # kT hi/lo [128, 2, 2048] per fhat-pair; v hi/lo [128, 16, 512]
            kh_p = [
                k_pool.tile([P, 2, S], F8, tag=f"kh{i}", name=f"kh{i}")
                for i in range(2)
            ]
            kl_p = [
                k_pool.tile([P, 2, S], F8, tag=f"kl{i}", name=f"kl{i}")
                for i in range(2)
            ]
            vh = v_pool.tile([P, NT, D], F8, tag="vh", name="vh")
            vl = v_pool.tile([P, NT, D], F8, tag="vl", name="vl")

            bias_sb = consts.tile([P, NT], F32, tag="bias", name="bias_sb")

            with (
                tc.tile_pool(name="psB", bufs=1, space="PSUM") as psB,
                tc.tile_pool(name="psProj", bufs=5, space="PSUM") as psProj,
            ):
                # ---- PE warmup: ~3.5us of junk matmuls while the x/M DMAs
                # land, so the PE p-state is at full clock for real work ----
                ps_w = psB.tile([16, 512], F32, tag="warm", name="ps_warm")
                for w in range(10):
                    nc.tensor.matmul(
                        ps_w[:],
                        warm8[:, :, 0:16],
                        warm8[:],
                        start=(w == 0),
                        stop=(w == 13),
                        perf_mode=DR,
                    )

                # ---- kT projection: kT = M'-slabs . x^T (3-combo fp8) ----
                for kc in range(4):
                    for ft in range(4):
                        ps = psProj.tile(
                            [P, 512], F32, tag="ps_p", name=f"ps_k{kc}_{ft}"
                        )
                        combos = (
                            (mh_p, xh_p),
                            (ml_p, xh_p),
                            (mh_p, xl_p),
                        )
                        n = 0
                        for wm, xm in combos:
                            for c2 in range(2):
                                nc.tensor.matmul(
                                    ps[:],
                                    wm[c2][:, :, ft * P : (ft + 1) * P],
                                    xm[c2][:, :, kc * 512 : (kc + 1) * 512],
                                    start=(n == 0),
                                    stop=(n == 5),
                                    perf_mode=DR,
                                )
                                n += 1
                        dst = (slice(None), ft % 2, slice(kc * 512, (kc + 1) * 512))
                        nc.scalar.copy(kh_p[ft // 2][dst], ps[:])
                        nc.vector.tensor_sub(
                            kl_p[ft // 2][dst], ps[:], kh_p[ft // 2][dst]
                        )

                # ---- v projection (3-combo fp8), bias bv at output ----
                # The per-key bias row b^T rides along: 1-column DoubleRow
                # matmuls reusing xh as stationary, m3 as moving.
                ps_b2 = psB.tile([P, NT], F32, tag="ps_b2", name="ps_b2")
                for st in range(NT):
                    ps = psProj.tile([P, 512], F32, tag="ps_p", name=f"ps_v{st}")
                    combos = (
                        (xh_p, wvh_p),
                        (xl_p, wvh_p),
                        (xh_p, wvl_p),
                    )
                    n = 0
                    for xm, wm in combos:
                        for c2 in range(2):
                            nc.tensor.matmul(
                                ps[:],
                                xm[c2][:, :, st * P : (st + 1) * P],
                                wm[c2][:],
                                start=(n == 0),
                                stop=(n == 5),
                                perf_mode=DR,
                            )
                            n += 1
                    # one accumulation group for the whole ps_b2 bank: start
                    # marks the full 2KB zero region, later columns overwrite
                    # their pending-zero bytes on first touch
                    for c2 in range(2):
                        nc.tensor.matmul(
                            ps_b2[:, st : st + 1],
                            xh_p[c2][:, :, st * P : (st + 1) * P],
                            m3t[c2][:],
                            start=(st == 0 and c2 == 0),
                            stop=(st == NT - 1 and c2 == 1),
                            perf_mode=DR,
                        )
                    nc.scalar.copy(vh[:, st, :], ps[:])
                    nc.vector.tensor_sub(vl[:, st, :], ps[:], vh[:, st, :])

                nc.scalar.activation(
                    bias_sb[:],
                    ps_b2[:],
                    ACT_ID,
                    bias=neg_shift[:],
                    scale=1.0 / A_3,
                )

            # ---- attention: scores^T -> exp -> e hi/lo -> den + attn*V ----
            # Software-pipelined: the attn*V + denominator matmuls for key
            # pair p are emitted AFTER the scores of pair p+1, so the PE
            # (in-order) never waits on the exp -> eh -> el chain (~2.4us).
            with (
                tc.tile_pool(name="psS", bufs=2, space="PSUM") as psS,
                tc.tile_pool(name="psO", bufs=1, space="PSUM") as psO,
                tc.tile_pool(name="psDen", bufs=1, space="PSUM") as psDen,
            ):
                ps_o_m = {}
                ps_den_m = {}
                pend = []

                def emit_av(m, p, eh_t, el_t):
                    ps_o = ps_o_m[m]
                    ps_den = ps_den_m[m]
                    vsl = slice(2 * p, 2 * p + 2)
                    last = p == NT // 2 - 1
                    den_qts = range(4) if last else ()
                    if last:
                        # final pair: denominators first so the reciprocal
                        # starts while the PE still runs the last AV matmuls
                        for qt in range(4):
                            qsl = slice(qt * P, (qt + 1) * P)
                            nc.tensor.matmul(
                                ps_den[:, qt : qt + 1], eh_t[:, :, qsl],
                                alpha8[:], start=False, stop=False,
                                perf_mode=DR,
                            )
                            nc.tensor.matmul(
                                ps_den[:, qt : qt + 1], el_t[:, :, qsl],
                                alpha8[:], start=False, stop=(qt == 3),
                                perf_mode=DR,
                            )
                        rec = den_pool.tile([P, 4], F32, tag="rec", name=f"rec{m}")
                        nc.vector.reciprocal(rec[:], ps_den[:])
                    for qt in range(4):
                        qsl = slice(qt * P, (qt + 1) * P)
                        for ci, (em, vm) in enumerate(
                            ((eh_t, vh), (el_t, vh), (eh_t, vl))
                        ):
                            nc.tensor.matmul(
                                ps_o[qt][:],
                                em[:, :, qsl],
                                vm[:, vsl, :],
                                start=(p == 0 and ci == 0),
                                stop=(last and ci == 2),
                                perf_mode=DR,
                            )
                        if not last:
                            # denominators ride the same stationaries, one
                            # psum group for the whole ps_den bank (start
                            # marks the full zero region; columns fill on
                            # first touch)
                            nc.tensor.matmul(
                                ps_den[:, qt : qt + 1],
                                eh_t[:, :, qsl],
                                alpha8[:],
                                start=(p == 0 and qt == 0),
                                stop=False,
                                perf_mode=DR,
                            )
                            nc.tensor.matmul(
                                ps_den[:, qt : qt + 1],
                                el_t[:, :, qsl],
                                alpha8[:],
                                start=False,
                                stop=False,
                                perf_mode=DR,
                            )
                        else:
                            # out = ps_o * (1/den) + bv, fused; DMA out
                            o_sb = out_pool.tile(
                                [P, D], F32, tag="osb", name=f"o{m}_{qt}"
                            )
                            nc.vector.scalar_tensor_tensor(
                                o_sb[:], ps_o[qt][:], rec[:, qt : qt + 1],
                                bv_sb[:], MULT, ADD,
                            )
                            it = m * 4 + qt
                            q = nc.sync if qt % 2 == 0 else nc.gpsimd
                            q.dma_start(
                                out=out[it * P : (it + 1) * P, :], in_=o_sb[:]
                            )

                for m in range(MC):
                    msl = slice(m * 512, (m + 1) * 512)
                    ps_o_m[m] = [
                        psO.tile([P, D], F32, tag=f"o{t}", name=f"ps_o{t}_{m}")
                        for t in range(4)
                    ]
                    # transposed denominators: [query 128, qt 4], scaled by A_V
                    ps_den_m[m] = psDen.tile(
                        [P, 4], F32, tag="ps_den", name=f"ps_den{m}"
                    )
                    eh_t = el_t = None
                    for kt in range(NT):
                        p = kt // 2
                        ps_s = psS.tile(
                            [P, 512], F32, tag="ps_s", name=f"ps_s{m}_{kt}"
                        )
                        ktsl = slice(kt * P, (kt + 1) * P)
                        combos = (
                            (kh_p, xh_p),
                            (kh_p, xl_p),
                            (kl_p, xh_p),
                        )
                        n = 0
                        for km, xm in combos:
                            for c2 in range(2):
                                nc.tensor.matmul(
                                    ps_s[:],
                                    km[c2][:, :, ktsl],
                                    xm[c2][:, :, msl],
                                    start=(n == 0),
                                    stop=(n == 5),
                                    perf_mode=DR,
                                )
                                n += 1
                        if kt % 2 == 0:
                            eh_t = e8_pool.tile(
                                [P, 2, 512], F8, tag="eh", name=f"eh{m}_{p}"
                            )
                            el_t = e8_pool.tile(
                                [P, 2, 512], F8, tag="el", name=f"el{m}_{p}"
                            )
                        e32 = e32_pool.tile(
                            [P, 512], F32, tag="e32", name=f"e32_{m}_{kt}"
                        )
                        nc.scalar.activation(
                            e32[:],
                            ps_s[:],
                            ACT_EXP,
                            bias=bias_sb[:, kt : kt + 1],
                            scale=1.0 / A_M,
                        )
                        # e hi/lo split: eh on Pool; el mostly on DVE with a
                        # quarter on Pool (Pool can't reach PSUM, so it only
                        # ever sees these SBUF-only ops)
                        nc.gpsimd.tensor_copy(eh_t[:, kt % 2, :], e32[:])
                        nc.vector.tensor_sub(
                            el_t[:, kt % 2, :], e32[:], eh_t[:, kt % 2, :]
                        )

                        if kt % 2 == 1:
                            pend.append((m, p, eh_t, el_t))
                            if len(pend) > 3:
                                emit_av(*pend.pop(0))
                for args in pend:
                    emit_av(*args)

    nc.finalize()
    return nc


def kernel(x, Wq, bq, Wk, bk, Wv, bv):
    x = np.asarray(x, dtype=np.float32)
    Wq = np.asarray(Wq, dtype=np.float32)
    bq = np.asarray(bq, dtype=np.float32)
    Wk = np.asarray(Wk, dtype=np.float32)
    bk = np.asarray(bk, dtype=np.float32)
    Wv = np.asarray(Wv, dtype=np.float32)
    bv = np.asarray(bv, dtype=np.float32)

    # host-side folding: stationary M' = Wk Wq^T * scale, m3 = Wk bq * scale
    mt = (
        (Wk.astype(np.float64) @ Wq.astype(np.float64).T) * SCALE * A_M
    ).astype(np.float32)
    mht, mlt = _split8(mt)
    wvh, wvl = _split8(Wv * np.float32(A_V))
    m3v = (
        (Wk.astype(np.float64) @ bq.astype(np.float64)) * SCALE * A_3
    ).astype(np.float32).astype(E4M3)
    m3 = m3v[:, None]

    shared = {
        "mht": np.ascontiguousarray(mht),
        "mlt": np.ascontiguousarray(mlt),
        "wvh": np.ascontiguousarray(wvh),
        "wvl": np.ascontiguousarray(wvl),
        "m3": np.ascontiguousarray(m3),
        "bv": np.ascontiguousarray(bv),
    }

    if "nc" not in _CACHE:
        _CACHE["nc"] = _build_nc()
    nc = _CACHE["nc"]

    in_maps = []
    for b in range(B):
        xT = np.ascontiguousarray(x[b].T)
        xhb, xlb = _split8(xT)
        in_maps.append(
            {"xh": np.ascontiguousarray(xhb), "xl": np.ascontiguousarray(xlb), **shared}
        )
    try:
        res = run_bass_kernel_spmd(nc, in_maps, list(range(B)))
    except Exception:
        # transient device wedge (e.g. NRT_EXEC_UNIT_UNRECOVERABLE) - retry
        import time as _time

        _time.sleep(5)
        res = run_bass_kernel_spmd(nc, in_maps, list(range(B)))
    return np.stack([res.results[b]["out"] for b in range(B)]).astype(np.float32)


if __name__ == "__main__":
    rng = np.random.default_rng(0)
    inputs = {
        "x": rng.standard_normal((B, S, D), dtype=np.float32),
        "Wq": rng.standard_normal((D, D), dtype=np.float32) / np.sqrt(D),
        "bq": rng.standard_normal(D).astype(np.float32) * 0.01,
        "Wk": rng.standard_normal((D, D), dtype=np.float32) / np.sqrt(D),
        "bk": rng.standard_normal(D).astype(np.float32) * 0.01,
        "Wv": rng.standard_normal((D, D), dtype=np.float32) / np.sqrt(D),
        "bv": rng.standard_normal(D).astype(np.float32) * 0.01,
    }
    got = kernel(**inputs)
    print("kernel output", got.shape, got.dtype)


# revision 57
# speedup vs baseline: 1.0146x; 1.0135x over previous
"""Trainium2 Bass kernel for single-head attention.

Problem: x[8, 2048, 512], Wq/Wk/Wv[512, 512], bq/bk/bv[512] ->
out[8, 2048, 512] where out = softmax((xWq+bq)(xWk+bk)^T / sqrt(512)) (xWv+bv).

Sharding: data-parallel over batch; each NeuronCore does one batch element.

Key ideas (vs the straightforward fp32r kernel):

1. Algebraic folding. q.k^T = x (Wq Wk^T) x^T + (x Wq).bk + bq.(x Wk) + bq.bk.
   Terms constant along the softmax (key) axis cancel in softmax, so only
   M' = Wq Wk^T * scale (host-precomputed) and the per-key bias
   b_j = x_j . (Wk bq * scale) survive. The Q and K projections collapse into
   ONE projection kT = M' x^T, and eT[j, i] = exp(x_i . kT_j + b_j - SHIFT).
   b_j rides in the ACT per-partition bias of the exp; the e^{-SHIFT} and
   per-query factors cancel in num/den.

2. fp8 DoubleRow matmuls. TRN2's fp8e4 DoubleRow mode runs 2 stacked
   128-contraction slabs per instruction at 0.5 cycles/row -> 4x fp32r
   MAC throughput. e4m3 alone is far too coarse (~2.5% per element), so every
   operand is split hi/lo: a = fp8(a) + fp8(a - fp8(a)), exact to ~7e-4, and
   each product uses 3 of the 4 cross terms (ah.bh + al.bh + ah.bl; the
   dropped al.bl is ~7e-4 relative). Host splits x^T, M', Wv exactly; the
   device splits its own intermediates (kT, v, e) with one fp8 copy + one
   subtract per tile, spread across ACT/DVE/Pool so no engine exceeds ~70us
   (Pool runs at 0.42-0.6 efficiency and cannot touch PSUM). e4m3's narrow
   range ([2^-9, 240]) forces power-of-2 pre-scales A_M/A_3/A_V on M'/m3/Wv,
   compensated for free in the exp scale, the bias-prep scale, and the
   fp8 value of the denominator 'ones' vector.

3. Denominators and the bias row are computed TRANSPOSED as 1-column
   DoubleRow matmuls whose stationary operands are shared with the adjacent
   attn*V / v-projection instructions, so they cost ~0 PE cycles and need no
   [1,512]->[128,..] transposes. bv is applied at output evacuation with a
   fused (psum * 1/den) + bv scalar_tensor_tensor. No PE transposes of x
   (host ships x^T pre-split).

Per-core matmul cycle budget (PE at 2.4GHz): kT-proj 24.6K, v-proj 24.6K,
scores 98.3K, attn*V 98.3K, den/bias ~0.2K => ~246K cycles ~= 103us, vs
379K (~158us) for the fp32r baseline. Measured rel err ~2.4e-3 (budget 2e-2).
"""

import os
import sys

for _p in ("/opt/trn_rl_repo", "/root/.axon_site/_ro/trn_rl_repo"):
    if os.path.isdir(_p) and _p not in sys.path:
        sys.path.append(_p)

import numpy as np
import ml_dtypes

import concourse.bacc as bacc
import concourse.mybir as mybir
import concourse.tile as tile
from concourse.bass_utils import run_bass_kernel_spmd

B = 8
S = 2048
D = 512
P = 128
NT = S // P  # 16 key tiles
MC = 4  # query chunks of 512
SCALE = 1.0 / float(np.sqrt(D))
SHIFT = 3.0  # keeps exp(score - SHIFT) well inside fp8e4 range (max ~240)
# Power-of-2 pre-scales keep each fp8 operand mid-range (e4m3 spans only
# [2^-9, 240]); they are exact in fp8 and compensated downstream: A_M in the
# exp's scale, A_3 in the bias prep, A_V in the den 'ones' fp8 value.
A_M = 512.0
A_3 = 8192.0
A_V = 32.0

F32 = mybir.dt.float32
F8 = mybir.dt.float8e4
E4M3 = ml_dtypes.float8_e4m3
ACT_EXP = mybir.ActivationFunctionType.Exp
ACT_ID = mybir.ActivationFunctionType.Identity
DR = mybir.MatmulPerfMode.DoubleRow
MULT = mybir.AluOpType.mult
ADD = mybir.AluOpType.add

_CACHE = {}


def _split8(a):
    """Split fp32 array into (hi, lo) e4m3 parts with hi + lo ~= a."""
    hi = a.astype(E4M3)
    lo = (a - hi.astype(np.float32)).astype(E4M3)
    return hi, lo


def _build_nc():
    nc = bacc.Bacc(None)

    xh = nc.dram_tensor("xh", [D, S], F8, kind="ExternalInput")
    xl = nc.dram_tensor("xl", [D, S], F8, kind="ExternalInput")
    mht = nc.dram_tensor("mht", [D, D], F8, kind="ExternalInput")
    mlt = nc.dram_tensor("mlt", [D, D], F8, kind="ExternalInput")
    wvh = nc.dram_tensor("wvh", [D, D], F8, kind="ExternalInput")
    wvl = nc.dram_tensor("wvl", [D, D], F8, kind="ExternalInput")
    m3 = nc.dram_tensor("m3", [D, 1], F8, kind="ExternalInput")
    bv = nc.dram_tensor("bv", [D], F32, kind="ExternalInput")
    out = nc.dram_tensor("out", [S, D], F32, kind="ExternalOutput")

    with tile.TileContext(nc) as tc:
        with (
            tc.tile_pool(name="consts", bufs=1) as consts,
            tc.tile_pool(name="xT", bufs=1) as xT_pool,
            tc.tile_pool(name="wsb", bufs=1) as w_pool,
            tc.tile_pool(name="ksb", bufs=1) as k_pool,
            tc.tile_pool(name="vsb", bufs=1) as v_pool,
            tc.tile_pool(name="e32", bufs=7) as e32_pool,
            tc.tile_pool(name="e8", bufs=7) as e8_pool,
            tc.tile_pool(name="den", bufs=2) as den_pool,
            tc.tile_pool(name="outsb", bufs=4) as out_pool,
        ):
            # ---- constants ----
            alpha32 = consts.tile([P, 2, 1], F32, tag="alpha32", name="alpha32")
            nc.vector.memset(alpha32[:], A_V)
            alpha8 = consts.tile([P, 2, 1], F8, tag="alpha8", name="alpha8")
            nc.gpsimd.tensor_copy(alpha8[:], alpha32[:])
            neg_shift = consts.tile([P, 1], F32, tag="nshift", name="neg_shift")
            nc.vector.memset(neg_shift[:], -SHIFT)

            warm8 = consts.tile([P, 2, 512], F8, tag="warm8", name="warm8")
            nc.gpsimd.memset(warm8[:], 1.0)

            bv_sb = consts.tile([P, D], F32, tag="bv", name="bv_sb")
            nc.gpsimd.dma_start(out=bv_sb[:], in_=bv[:].partition_broadcast(P))

            # m3 moving tiles [f' 128, 2, 1] per f'-pair
            m3t = []
            for c2 in range(2):
                t = consts.tile([P, 2, 1], F8, tag=f"m3_{c2}", name=f"m3_{c2}")
                nc.gpsimd.dma_start(
                    out=t[:],
                    in_=m3[c2 * 256 : (c2 + 1) * 256, :].rearrange(
                        "(two p) c -> p two c", p=P
                    ),
                )
                m3t.append(t)

            # weights: stationary/moving tiles [128, 2, 512] per f'-pair.
            # mh goes on the sync queue (needed first); the rest via gpsimd.
            def _wtiles(name, dram, eng):
                ts = []
                for c2 in range(2):
                    t = w_pool.tile(
                        [P, 2, D], F8, tag=f"{name}{c2}", name=f"{name}{c2}"
                    )
                    eng.dma_start(
                        out=t[:],
                        in_=dram[c2 * 256 : (c2 + 1) * 256, :].rearrange(
                            "(two p) h -> p two h", p=P
                        ),
                    )
                    ts.append(t)
                return ts

            mh_p = _wtiles("mh", mht, nc.scalar)
            ml_p = _wtiles("ml", mlt, nc.scalar)

            # x^T hi/lo, [128, 2, 2048] per f-pair. DMA'd in key-chunk
            # slices, first chunk of every tile first, so the kT projection
            # (which walks key-chunks) unblocks after ~1/4 of the bytes.
            def _xalloc(name):
                return [
                    xT_pool.tile([P, 2, S], F8, tag=f"{name}{c2}", name=f"{name}{c2}")
                    for c2 in range(2)
                ]

            xh_p = _xalloc("xh")
            xl_p = _xalloc("xl")
            for kc in range(4):
                ksl = slice(kc * 512, (kc + 1) * 512)
                for tiles, dram, q in ((xl_p, xl, nc.gpsimd), (xh_p, xh, nc.sync)):
                    for c2 in range(2):
                        q.dma_start(
                            out=tiles[c2][:, :, ksl],
                            in_=dram[c2 * 256 : (c2 + 1) * 256, ksl].rearrange(
                                "(two p) s -> p two s", p=P
                            ),
                        )

            # remaining weights follow the xl slices on the gpsimd queue
            wvh_p = _wtiles("wvh", wvh, nc.gpsimd)
            wvl_p = _wtiles("wvl", wvl, nc.gpsimd)

            # kT hi/lo [128, 2, 2048] per fhat-pair; v hi/lo [128, 16, 512]
            kh_p = [
                k_pool.tile([P, 2, S], F8, tag=f"kh{i}", name=f"kh{i}")
                for i in range(2)
            ]
            kl_p = [
                k_pool.tile([P, 2, S], F8, tag=f"kl{i}", name=f"kl{i}")
                for i in range(2)
            ]
            vh = v_pool.tile([P, NT, D], F8, tag="vh", name="vh")
            vl = v_pool.tile([P, NT, D], F8, tag="vl", name="vl")

            bias_sb = consts.tile([P, NT], F32, tag="bias", name="bias_sb")

            with (
                tc.tile_pool(name="psB", bufs=1, space="PSUM") as psB,
                tc.tile_pool(name="psProj", bufs=5, space="PSUM") as psProj,
            ):
                # ---- PE warmup: ~3.5us of junk matmuls while the x/M DMAs
                # land, so the PE p-state is at full clock for real work ----
                ps_w = psB.tile([16, 512], F32, tag="warm", name="ps_warm")
                for w in range(10):
                    nc.tensor.matmul(
                        ps_w[:],
                        warm8[:, :, 0:16],
                        warm8[:],
                        start=(w == 0),
                        stop=(w == 13),
                        perf_mode=DR,
                    )

                # ---- kT projection: kT = M'-slabs . x^T (3-combo fp8) ----
                for kc in range(4):
                    for ft in range(4):
                        ps = psProj.tile(
                            [P, 512], F32, tag="ps_p", name=f"ps_k{kc}_{ft}"
                        )
                        combos = (
                            (mh_p, xh_p),
                            (ml_p, xh_p),
                            (mh_p, xl_p),
                        )
                        n = 0
                        for wm, xm in combos:
                            for c2 in range(2):
                                nc.tensor.matmul(
                                    ps[:],
                                    wm[c2][:, :, ft * P : (ft + 1) * P],
                                    xm[c2][:, :, kc * 512 : (kc + 1) * 512],
                                    start=(n == 0),
                                    stop=(n == 5),
                                    perf_mode=DR,
                                )
                                n += 1
                        dst = (slice(None), ft % 2, slice(kc * 512, (kc + 1) * 512))
                        nc.scalar.copy(kh_p[ft // 2][dst], ps[:])
                        nc.vector.tensor_sub(
                            kl_p[ft // 2][dst], ps[:], kh_p[ft // 2][dst]
                        )

                # ---- v projection (3-combo fp8), bias bv at output ----
                # The per-key bias row b^T rides along: 1-column DoubleRow
                # matmuls reusing xh as stationary, m3 as moving.
                ps_b2 = psB.tile([P, NT], F32, tag="ps_b2", name="ps_b2")
                for st in range(NT):
                    ps = psProj.tile([P, 512], F32, tag="ps_p", name=f"ps_v{st}")
                    combos = (
                        (xh_p, wvh_p),
                        (xl_p, wvh_p),
                        (xh_p, wvl_p),
                    )
                    n = 0
                    for xm, wm in combos:
                        for c2 in range(2):
                            nc.tensor.matmul(
                                ps[:],
                                xm[c2][:, :, st * P : (st + 1) * P],
                                wm[c2][:],
                                start=(n == 0),
                                stop=(n == 5),
                                perf_mode=DR,
                            )
                            n += 1
                    # one accumulation group for the whole ps_b2 bank: start
                    # marks the full 2KB zero region, later columns overwrite
                    # their pending-zero bytes on first touch
                    for c2 in range(2):
                        nc.tensor.matmul(
                            ps_b2[:, st : st + 1],
                            xh_p[c2][:, :, st * P : (st + 1) * P],
                            m3t[c2][:],
                            start=(st == 0 and c2 == 0),
                            stop=(st == NT - 1 and c2 == 1),
                            perf_mode=DR,
                        )
                    nc.scalar.copy(vh[:, st, :], ps[:])
                    nc.vector.tensor_sub(vl[:, st, :], ps[:], vh[:, st, :])

                nc.scalar.activation(
                    bias_sb[:],
                    ps_b2[:],
                    ACT_ID,
                    bias=neg_shift[:],
                    scale=1.0 / A_3,
                )

            # ---- attention: scores^T -> exp -> e hi/lo -> den + attn*V ----
            # Software-pipelined: the attn*V + denominator matmuls for key
            # pair p are emitted AFTER the scores of pair p+1, so the PE
            # (in-order) never waits on the exp -> eh -> el chain (~2.4us).
            with (
                tc.tile_pool(name="psS", bufs=2, space="PSUM") as psS,
                tc.tile_pool(name="psO", bufs=1, space="PSUM") as psO,
                tc.tile_pool(name="psDen", bufs=1, space="PSUM") as psDen,
            ):
                ps_o_m = {}
                ps_den_m = {}
                pend = []

                def emit_av(m, p, eh_t, el_t):
                    ps_o = ps_o_m[m]
                    ps_den = ps_den_m[m]
                    vsl = slice(2 * p, 2 * p + 2)
                    last = p == NT // 2 - 1
                    den_qts = range(4) if last else ()
                    if last:
                        # final pair: denominators first so the reciprocal
                        # starts while the PE still runs the last AV matmuls
                        for qt in range(4):
                            qsl = slice(qt * P, (qt + 1) * P)
                            nc.tensor.matmul(
                                ps_den[:, qt : qt + 1], eh_t[:, :, qsl],
                                alpha8[:], start=False, stop=False,
                                perf_mode=DR,
                            )
                            nc.tensor.matmul(
                                ps_den[:, qt : qt + 1], el_t[:, :, qsl],
                                alpha8[:], start=False, stop=(qt == 3),
                                perf_mode=DR,
                            )
                        rec = den_pool.tile([P, 4], F32, tag="rec", name=f"rec{m}")
                        nc.vector.reciprocal(rec[:], ps_den[:])
                    for qt in range(4):
                        qsl = slice(qt * P, (qt + 1) * P)
                        for ci, (em, vm) in enumerate(
                            ((eh_t, vh), (el_t, vh), (eh_t, vl))
                        ):
                            nc.tensor.matmul(
                                ps_o[qt][:],
                                em[:, :, qsl],
                                vm[:, vsl, :],
                                start=(p == 0 and ci == 0),
                                stop=(last and ci == 2),
                                perf_mode=DR,
                            )
                        if not last:
                            # denominators ride the same stationaries, one
                            # psum group for the whole ps_den bank (start
                            # marks the full zero region; columns fill on
                            # first touch)
                            nc.tensor.matmul(
                                ps_den[:, qt : qt + 1],
                                eh_t[:, :, qsl],
                                alpha8[:],
                                start=(p == 0 and qt == 0),
                                stop=False,
                                perf_mode=DR,
                            )
                            nc.tensor.matmul(
                                ps_den[:, qt : qt + 1],
                                el_t[:, :, qsl],
                                alpha8[:],
                                start=False,
                                stop=False,
                                perf_mode=DR,
                            )
                        else:
                            # out = ps_o * (1/den) + bv, fused; DMA out
                            o_sb = out_pool.tile(
                                [P, D], F32, tag="osb", name=f"o{m}_{qt}"
                            )
                            nc.vector.scalar_tensor_tensor(
                                o_sb[:], ps_o[qt][:], rec[:, qt : qt + 1],
                                bv_sb[:], MULT, ADD,
                            )
                            it = m * 4 + qt
                            q = nc.sync if qt % 2 == 0 else nc.gpsimd
                            q.dma_start(
                                out=out[it * P : (it + 1) * P, :], in_=o_sb[:]
                            )

                for m in range(MC):
                    msl = slice(m * 512, (m + 1) * 512)
                    ps_o_m[m] = [
                        psO.tile([P, D], F32, tag=f"o{t}", name=f"ps_o{t}_{m}")
                        for t in range(4)
                    ]
                    # transposed denominators: [query 128, qt 4], scaled by A_V
                    ps_den_m[m] = psDen.tile(
                        [P, 4], F32, tag="ps_den", name=f"ps_den{m}"
                    )
                    eh_t = el_t = None
                    for kt in range(NT):
                        p = kt // 2
                        ps_s = psS.tile(
                            [P, 512], F32, tag="ps_s", name=f"ps_s{m}_{kt}"
                        )
                        ktsl = slice(kt * P, (kt + 1) * P)
                        combos = (
                            (kh_p, xh_p),
                            (kh_p, xl_p),
                            (kl_p, xh_p),
                        )
                        n = 0
                        for km, xm in combos:
                            for c2 in range(2):
                                nc.tensor.matmul(
                                    ps_s[:],
                                    km[c2][:, :, ktsl],
                                    xm[c2][:, :, msl],
                                    start=(n == 0),
                                    stop=(n == 5),
                                    perf_mode=DR,
                                )
                                n += 1
                        if kt % 2 == 0:
                            eh_t = e8_pool.tile(
                                [P, 2, 512], F8, tag="eh", name=f"eh{m}_{p}"
                            )
                            el_t = e8_pool.tile(
                                [P, 2, 512], F8, tag="el", name=f"el{m}_{p}"
                            )
                        e32 = e32_pool.tile(
                            [P, 512], F32, tag="e32", name=f"e32_{m}_{kt}"
                        )
                        nc.scalar.activation(
                            e32[:],
                            ps_s[:],
                            ACT_EXP,
                            bias=bias_sb[:, kt : kt + 1],
                            scale=1.0 / A_M,
                        )
                        # e hi/lo split: eh on Pool; el mostly on DVE with a
                        # quarter on Pool (Pool can't reach PSUM, so it only
                        # ever sees these SBUF-only ops)
                        nc.gpsimd.tensor_copy(eh_t[:, kt % 2, :], e32[:])
                        nc.vector.tensor_sub(
                            el_t[:, kt % 2, :], e32[:], eh_t[:, kt % 2, :]
                        )

                        if kt % 2 == 1:
                            pend.append((m, p, eh_t, el_t))
                            if len(pend) > 3:
                                emit_av(*pend.pop(0))
                for args in pend:
                    emit_av(*args)

    nc.finalize()
    return nc


def kernel(x, Wq, bq, Wk, bk, Wv, bv):
    x = np.asarray(x, dtype=np.float32)
    Wq = np.asarray(Wq, dtype=np.float32)
    bq = np.asarray(bq, dtype=np.float32)
    Wk = np.asarray(Wk, dtype=np.float32)
    bk = np.asarray(bk, dtype=np.float32)
    Wv = np.asarray(Wv, dtype=np.float32)
    bv = np.asarray(bv, dtype=np.float32)

    # host-side folding: stationary M' = Wk Wq^T * scale, m3 = Wk bq * scale
    mt = (
        (Wk.astype(np.float64) @ Wq.astype(np.float64).T) * SCALE * A_M
    ).astype(np.float32)
    mht, mlt = _split8(mt)
    wvh, wvl = _split8(Wv * np.float32(A_V))
    m3v = (
        (Wk.astype(np.float64) @ bq.astype(np.float64)) * SCALE * A_3
    ).astype(np.float32).astype(E4M3)
    m3 = m3v[:, None]

    shared = {
        "mht": np.ascontiguousarray(mht),
        "mlt": np.ascontiguousarray(mlt),
        "wvh": np.ascontiguousarray(wvh),
        "wvl": np.ascontiguousarray(wvl),
        "m3": np.ascontiguousarray(m3),
        "bv": np.ascontiguousarray(bv),
    }

    if "nc" not in _CACHE:
        _CACHE["nc"] = _build_nc()
    nc = _CACHE["nc"]

    in_maps = []
    for b in range(B):
        xT = np.ascontiguousarray(x[b].T)
        xhb, xlb = _split8(xT)
        in_maps.append(
            {"xh": np.ascontiguousarray(xhb), "xl": np.ascontiguousarray(xlb), **shared}
        )
    try:
        res = run_bass_kernel_spmd(nc, in_maps, list(range(B)))
    except Exception:
        # transient device wedge (e.g. NRT_EXEC_UNIT_UNRECOVERABLE) - retry
        import time as _time

        _time.sleep(5)
        res = run_bass_kernel_spmd(nc, in_maps, list(range(B)))
    return np.stack([res.results[b]["out"] for b in range(B)]).astype(np.float32)


if __name__ == "__main__":
    rng = np.random.default_rng(0)
    inputs = {
        "x": rng.standard_normal((B, S, D), dtype=np.float32),
        "Wq": rng.standard_normal((D, D), dtype=np.float32) / np.sqrt(D),
        "bq": rng.standard_normal(D).astype(np.float32) * 0.01,
        "Wk": rng.standard_normal((D, D), dtype=np.float32) / np.sqrt(D),
        "bk": rng.standard_normal(D).astype(np.float32) * 0.01,
        "Wv": rng.standard_normal((D, D), dtype=np.float32) / np.sqrt(D),
        "bv": rng.standard_normal(D).astype(np.float32) * 0.01,
    }
    got = kernel(**inputs)
    print("kernel output", got.shape, got.dtype)
